# revision 2
# baseline (speedup 1.0000x reference)
"""ColumnParallelLinearWithMoE Trainium2 kernel.

Expert-parallel: expert e -> NeuronCore e. Each core computes
    y_e [8192, 512] = x_e [8192, 1024] @ W_e.T [1024, 512] + b_e
where x_e = input_[idx_list[e]] flattened over (per, seq).

Routing gather/scatter and the x transpose (to put the contraction dim on
SBUF partitions) happen on the host; the device does the dense matmul.
"""

import sys

if "/opt/trn_rl_repo" not in sys.path:
    sys.path.insert(0, "/opt/trn_rl_repo")

import numpy as np

# Problem constants (hardcoded per harness contract).
E = 8
BS = 64
S = 1024
D = 1024
OPP = 512
P = 128
TOK = (BS // E) * S  # 8192 tokens per expert
KT = D // P          # 8 contraction tiles
TW = 1024            # token-superblock width staged in SBUF
NSUP = TOK // TW
TPS = TW // P        # token tiles (of 128) per superblock

# Matmul dtype variant: "f32" (exact, slow), "f32r" (fast fp32 path),
# "bf16" (host-cast, fastest DMA).
VARIANT = "f32"

_programs: dict[str, tuple] = {}


def _build(variant: str):
    import concourse.bacc as bacc
    import concourse.tile as tile
    from concourse import mybir

    if variant == "f32":
        mm_dt = mybir.dt.float32
        np_in = np.float32
    elif variant == "f32r":
        mm_dt = mybir.dt.float32r
        np_in = np.float32
    elif variant == "bf16":
        import ml_dtypes

        mm_dt = mybir.dt.bfloat16
        np_in = ml_dtypes.bfloat16
    else:
        raise ValueError(variant)

    nc = bacc.Bacc(None, target_bir_lowering=False, debug=False)

    xt = nc.dram_tensor("xt", [D, TOK], mm_dt, kind="ExternalInput")
    wt = nc.dram_tensor("wt", [D, OPP], mm_dt, kind="ExternalInput")
    bias = nc.dram_tensor("bias", [P, OPP], mybir.dt.float32, kind="ExternalInput")
    y = nc.dram_tensor("y", [TOK, OPP], mybir.dt.float32, kind="ExternalOutput")

    with tile.TileContext(nc) as tc:
        with (
            tc.tile_pool(name="wpool", bufs=1) as wpool,
            tc.tile_pool(name="bpool", bufs=1) as bpool,
            tc.tile_pool(name="xpool", bufs=2) as xpool,
            tc.tile_pool(name="opool", bufs=4) as opool,
            tc.tile_pool(name="pspool", bufs=4, space="PSUM") as pspool,
        ):
            bias_sb = bpool.tile([P, OPP], mybir.dt.float32)
            nc.sync.dma_start(out=bias_sb[:], in_=bias[:])

            w_sb = []
            for k in range(KT):
                wtile = wpool.tile([P, OPP], mm_dt, tag=f"w{k}")
                nc.sync.dma_start(out=wtile[:], in_=wt[k * P : (k + 1) * P, :])
                w_sb.append(wtile)

            for s in range(NSUP):
                x_sb = []
                for k in range(KT):
                    xtile = xpool.tile([P, TW], mm_dt, tag=f"x{k}")
                    nc.sync.dma_start(
                        out=xtile[:],
                        in_=xt[k * P : (k + 1) * P, s * TW : (s + 1) * TW],
                    )
                    x_sb.append(xtile)
                for j in range(TPS):
                    ps = pspool.tile([P, OPP], mybir.dt.float32)
                    for k in range(KT):
                        nc.tensor.matmul(
                            ps[:],
                            x_sb[k][:, j * P : (j + 1) * P],
                            w_sb[k][:],
                            start=(k == 0),
                            stop=(k == KT - 1),
                        )
                    o_sb = opool.tile([P, OPP], mybir.dt.float32)
                    nc.vector.tensor_add(o_sb[:], ps[:], bias_sb[:])
                    t = s * TPS + j
                    nc.sync.dma_start(
                        out=y[t * P : (t + 1) * P, :], in_=o_sb[:]
                    )

    nc.compile()
    return nc, np_in


def _get_program(variant: str):
    if variant not in _programs:
        _programs[variant] = _build(variant)
    return _programs[variant]


def kernel(input_, idx_list, W, b, **_ignored):
    from concourse.bass_utils import run_bass_kernel_spmd

    input_ = np.asarray(input_)
    idx = np.asarray(idx_list).astype(np.int64)
    W = np.asarray(W, dtype=np.float32)
    b = np.asarray(b, dtype=np.float32)

    nc, np_in = _get_program(VARIANT)

    in_maps = []
    for e in range(E):
        xg = input_[idx[e]].reshape(TOK, D).astype(np.float32, copy=False)
        xtr = np.ascontiguousarray(xg.T).astype(np_in)
        wtr = np.ascontiguousarray(W[e].T).astype(np_in)
        bias_bc = np.ascontiguousarray(
            np.broadcast_to(b[e][None, :], (P, OPP))
        ).astype(np.float32)
        in_maps.append({"xt": xtr, "wt": wtr, "bias": bias_bc})

    res = run_bass_kernel_spmd(nc, in_maps, core_ids=list(range(E)))

    out = np.zeros((BS, S, E * OPP), dtype=input_.dtype)
    for e in range(E):
        ye = np.asarray(res.results[e]["y"], dtype=input_.dtype)
        out[idx[e], :, e * OPP : (e + 1) * OPP] = ye.reshape(BS // E, S, OPP)
    return out
